# revision 32
# baseline (speedup 1.0000x reference)
"""Causal self-attention Trainium2 Bass kernel (v2).

Problem: B=2, T=4096, C=512, H=8 heads, D=64.
  q = x@Wq.T, k = x@Wk.T, v = x@Wv.T  (per-head split)
  att = softmax(causal(q k^T / sqrt(D)));  y = att @ v;  out = y @ Wout.T

Sharding: 8 cores = 2 batches x 4 head-groups (2 heads each).
Each core computes, for its batch b and heads {2g, 2g+1}:
  - feature-major qT,kT [128, T] bf16 and augmented v blocks via PE matmuls
  - per t1-chunk (512 wide): transposed scores ST[t2_block, t1] = kT^T qT in
    bf16; diagonal blocks stream only the valid causal suffix and get a
    constant 128x128 corner mask added on DVE; exp on ACT (scale=1/8,
    strided suffix AP) into bf16 et tiles; yT_aug[65, t1] accumulation with
    a ones column so row 64 = softmax denominator.
  - tail: denominators -> reciprocal_approx_fast -> broadcast via tiny K=2
    PE matmul, normalize, out = y_norm @ Wout[:, cols]^T -> [T, 512].
  - QKV for chunk c+1 and tail for c-1 are emitted inside chunk c's j-loop
    so the ACT engine (exp, the bottleneck) never drains.
Host sums the 4 partial outputs per batch (row-parallel out projection).
"""

import os
import sys

import numpy as np

B, T, C = 2, 4096, 512
H, D = 8, 64
P = 128          # partitions / t2-block size
CH = 512         # t1 chunk width
NCH = T // CH    # 8 chunks
NTB = T // P     # 32 t-blocks
KC = C // P      # 4 contraction chunks for projections
NEG = -1.0e5     # causal mask additive value (pre-scale)

_COMPILED = None


def _import_concourse():
    try:
        import concourse.bass  # noqa: F401
    except ImportError:
        for p in ("/opt/trn_rl_repo", os.path.expanduser("~/.axon_site/_ro/trn_rl_repo")):
            if os.path.isdir(p) and p not in sys.path:
                sys.path.insert(0, p)
        import concourse.bass  # noqa: F401


def _build():
    """Build + compile the SPMD Bass program (same program on all 8 cores)."""
    _import_concourse()
    import concourse.bass as bass  # noqa: F401
    import concourse.tile as tile
    from concourse import bacc, mybir

    f32 = mybir.dt.float32
    f32r = mybir.dt.float32r
    bf16 = mybir.dt.bfloat16
    EXP = mybir.ActivationFunctionType.Exp

    nc = bacc.Bacc("TRN2", target_bir_lowering=False, debug=False, num_devices=8)

    xT_d = nc.dram_tensor("xT", [C, T], bf16, kind="ExternalInput").ap()
    wq_d = nc.dram_tensor("wq", [P, C], bf16, kind="ExternalInput").ap()
    wk_d = nc.dram_tensor("wk", [P, C], bf16, kind="ExternalInput").ap()
    wv_d = nc.dram_tensor("wv", [P, C], bf16, kind="ExternalInput").ap()
    wo_d = nc.dram_tensor("wo", [P, C], bf16, kind="ExternalInput").ap()
    mkc_d = nc.dram_tensor("mkc", [P, 2 * P], f32, kind="ExternalInput").ap()
    sel_d = nc.dram_tensor("sel", [65, P], f32r, kind="ExternalInput").ap()
    id_d = nc.dram_tensor("idm", [P, P], f32r, kind="ExternalInput").ap()
    out_d = nc.dram_tensor("out", [T, C], bf16, kind="ExternalOutput").ap()

    import contextlib

    with tile.TileContext(nc) as tc, contextlib.ExitStack() as _pctx:
        # ---- persistent SBUF tensors
        persist = _pctx.enter_context(tc.tile_pool(name="persist", bufs=1))

        def ptile(shape, name, dt=f32):
            return persist.tile(shape, dt, name=name, tag=name)

        xT_sb = ptile([P, KC * T], "xT_sb", bf16)      # 4 MB
        wq_sb = ptile([P, C], "wq_sb", bf16)
        wk_sb = ptile([P, C], "wk_sb", bf16)
        wv_sb = ptile([P, C], "wv_sb", bf16)
        wo_sb = ptile([P, C], "wo_sb", bf16)
        mkc_sb = ptile([P, 2 * P], "mkc_sb", f32)
        sel_sb = ptile([65, P], "sel_sb", f32r)
        id_sb = ptile([P, P], "id_sb", f32r)
        qT_sb = ptile([P, T], "qT_sb", bf16)
        kT_sb = ptile([P, T], "kT_sb", bf16)
        va1_sb = ptile([P, NTB * 65], "va1_sb", bf16)
        va2_sb = ptile([P, NTB * 65], "va2_sb", bf16)
        yT_sb = ptile([P, T], "yT_sb", bf16)
        spair_sb = ptile([65, CH], "spair_sb", f32)  # rows 0/64 live, rest 1.0

        # ---- input DMAs ordered by first use: qkv(0) needs wq/wk/wv + xT
        # chunk 0; the first (diagonal) score block needs mkc; id for the v
        # transposes; sel/wo only at the first tail (~30us in).
        def dma_x(c, k):
            eng = nc.sync if (c + k) % 2 == 0 else nc.gpsimd
            eng.dma_start(
                xT_sb[:, T * k + CH * c : T * k + CH * (c + 1)],
                xT_d[P * k : P * (k + 1), CH * c : CH * (c + 1)],
            )

        nc.sync.dma_start(wk_sb[:], wk_d[:])
        nc.gpsimd.dma_start(wq_sb[:], wq_d[:])
        for k in range(KC):
            dma_x(0, k)
        nc.gpsimd.dma_start(wv_sb[:], wv_d[:])
        nc.sync.dma_start(mkc_sb[:], mkc_d[:])
        nc.gpsimd.dma_start(id_sb[:], id_d[:])
        for k in range(KC):
            dma_x(1, k)
        nc.sync.dma_start(sel_sb[:], sel_d[:])
        nc.gpsimd.dma_start(wo_sb[:], wo_d[:])
        for c in range(2, NCH):
            for k in range(KC):
                dma_x(c, k)

        ones_sb = ptile([P, NTB], "ones_sb")  # f32 staging for vaug ones cols
        nc.vector.memset(ones_sb[:], 1.0)
        nc.vector.memset(spair_sb[:], 1.0)
        warm_sb = ptile([P, NTB], "warm_sb")
        nc.scalar.activation(warm_sb[:], ones_sb[:], EXP, scale=0.125)
        ones_src = ones_sb[:].rearrange("p (b s) -> p b s", s=1)
        for va in (va1_sb, va2_sb):
            dst = va[:].rearrange("p (b s) -> p b s", s=65)[:, :, 64:65]
            nc.vector.tensor_copy(dst, ones_src)

        # ---- pools (PSUM: 2*2 + 2*1 + 2*1 = 8 banks)
        with contextlib.ExitStack() as ctx:
            ps_st = ctx.enter_context(tc.tile_pool(name="ps_st", bufs=2, space="PSUM"))
            ps_yt = ctx.enter_context(tc.tile_pool(name="ps_yt", bufs=2, space="PSUM"))
            ps_ms = ctx.enter_context(tc.tile_pool(name="ps_ms", bufs=2, space="PSUM"))
            sb_e = ctx.enter_context(tc.tile_pool(name="sb_e", bufs=8))
            sb_vt = ctx.enter_context(tc.tile_pool(name="sb_vt", bufs=2))
            sb_sm = ctx.enter_context(tc.tile_pool(name="sb_sm", bufs=2))
            sb_ob = ctx.enter_context(tc.tile_pool(name="sb_ob", bufs=6))

            def emit_qkv(c, fast_start=False, parts="kqv"):
                """fast_start (chunk 0 only): compute kT block 0 and qT
                first in narrow pieces so st(0,0) unblocks ~4us earlier
                during the cold-start window. parts selects "kq"/"v" so the
                two halves can be emitted at different points (spreading
                their DVE casts across the chunk)."""
                plan = []
                if "k" in parts:
                    plan += [(wk_sb, "k"), (wq_sb, "q")]
                if "v" in parts:
                    plan += [(wv_sb, "v")]
                if fast_start:
                    plan = [(wk_sb, "k0"), (wq_sb, "q"), (wk_sb, "k1"), (wv_sb, "v")]
                for w_sb, kind in plan:
                    lo, hi = 0, CH
                    if kind == "k0":
                        hi = P
                    elif kind == "k1":
                        lo = P
                    acc = ps_ms.tile([P, CH], f32, tag="ms", name=f"ps_{kind}{c}")
                    for k in range(KC):
                        nc.tensor.matmul(
                            acc[:, lo:hi],
                            w_sb[:, P * k : P * (k + 1)],
                            xT_sb[:, T * k + CH * c + lo : T * k + CH * c + hi],
                            start=(k == 0),
                            stop=(k == KC - 1),
                        )
                    if kind.startswith("k"):
                        nc.vector.tensor_copy(
                            kT_sb[:, CH * c + lo : CH * c + hi], acc[:, lo:hi]
                        )
                    elif kind == "q":
                        nc.vector.tensor_copy(qT_sb[:, CH * c : CH * (c + 1)], acc[:])
                    else:
                        vt = sb_vt.tile([P, CH], f32r, tag="vt", name=f"vt{c}")
                        nc.vector.tensor_copy(vt[:], acc[:])
                        for b2 in range(4):
                            jb = 4 * c + b2
                            tr = ps_ms.tile([P, CH], f32r, tag="ms", name=f"tr{jb}")
                            nc.tensor.transpose(
                                tr[:, 0:P], vt[:, P * b2 : P * (b2 + 1)], id_sb[:]
                            )
                            nc.vector.tensor_copy(
                                va1_sb[:, 65 * jb : 65 * jb + 64], tr[:, 0:64]
                            )
                            nc.vector.tensor_copy(
                                va2_sb[:, 65 * jb : 65 * jb + 64], tr[:, 64:P]
                            )

            def emit_st(c, j):
                """Scores for t2-block j against chunk c's queries.

                Diagonal blocks (j = 4c+r) stream only the valid causal
                suffix [128r:512] and get the constant corner mask added.
                Returns (et tile, r) where r is the suffix offset blocks.
                High priority: these (plus the mask adds and exps) feed ACT,
                the bottleneck engine — the scheduler should always prefer
                them over qkv/tail/yT work when ready.
                """
                with tc.high_priority():
                    return _emit_st(c, j)

            def _emit_st(c, j):
                r = j - 4 * c if j >= 4 * c else 0
                w = CH - P * r  # valid t1 width
                stp = ps_st.tile([P, 2 * CH], f32, tag="st", name=f"st{c}_{j}")
                for h in (0, 1):
                    nc.tensor.matmul(
                        stp[:, CH * h + P * r : CH * (h + 1)],
                        kT_sb[64 * h : 64 * (h + 1), P * j : P * (j + 1)],
                        qT_sb[64 * h : 64 * (h + 1), CH * c + P * r : CH * (c + 1)],
                        start=True,
                        stop=True,
                    )
                if j >= 4 * c:  # diagonal: corner mask on [128, 2, 128]
                    st3 = stp[:].rearrange("p (h w) -> p h w", h=2)
                    nc.vector.tensor_add(
                        st3[:, :, P * r : P * r + P],
                        st3[:, :, P * r : P * r + P],
                        mkc_sb[:].rearrange("p (h w) -> p h w", h=2),
                    )
                et = sb_e.tile([P, 2 * CH], bf16, tag="e", name=f"e{c}_{j}")
                src = stp[:].rearrange("p (h w) -> p h w", h=2)[:, :, P * r :]
                dst = et[:].rearrange("p (h w) -> p h w", h=2)[:, :, P * r :]
                nc.scalar.activation(dst, src, EXP, scale=0.125)
                return et, r

            def emit_yt(c, j, et, r, yts, first, last):
                yt1, yt2 = yts
                for h, yt in ((0, yt1), (1, yt2)):
                    nc.tensor.matmul(
                        yt[:, P * r :],
                        (va1_sb if h == 0 else va2_sb)[:, 65 * j : 65 * j + 65],
                        et[:, CH * h + P * r : CH * (h + 1)],
                        start=first,
                        stop=last,
                    )

            def emit_tail_dve(c, yts, last=False):
                """yt-psum-releasing copies + per-block reciprocals.

                Emitted right after the first ST of the next chunk so the yt
                banks free early (unblocking that chunk's first yT matmul)
                and the reciprocals run on DVE ahead of the qkv casts.
                For the last chunk the copies run on ACT (idle by then).
                """
                yt1, yt2 = yts
                cp = nc.scalar.copy if last else nc.vector.tensor_copy
                cp(spair_sb[0:1, :], yt1[64:65, :])
                cp(spair_sb[64:65, :], yt2[64:65, :])
                cp(yT_sb[0:64, CH * c : CH * (c + 1)], yt1[0:64, :])
                cp(yT_sb[64:P, CH * c : CH * (c + 1)], yt2[0:64, :])
                return sb_sm.tile([65, CH], f32r, tag="rp", name=f"rp{c}")

            def emit_tail_block(c, b2, rpair, last=False):
                """One 128-query block of the tail: reciprocal, denominator
                broadcast, normalize, out projection, drain, DMA. Blocks are
                emitted at spread-out j slots so their DVE work doesn't
                clump and starve the corner-mask adds the exps need. For the
                last chunk the psum drains run on ACT (idle by then)."""
                tb = 4 * c + b2
                sl = slice(P * b2, P * (b2 + 1))
                with nc.allow_low_precision("f32r reciprocal for softmax norm"):
                    nc.vector.reciprocal(rpair[:, sl], spair_sb[:, sl])
                rb = ps_ms.tile([P, P], f32, tag="ms", name=f"rb{tb}")
                nc.tensor.matmul(rb[:], sel_sb[:], rpair[:, sl], start=True, stop=True)
                nc.vector.tensor_mul(
                    yT_sb[:, P * tb : P * (tb + 1)],
                    yT_sb[:, P * tb : P * (tb + 1)],
                    rb[:],
                )
                op = ps_ms.tile([P, CH], f32, tag="ms", name=f"op{tb}")
                nc.tensor.matmul(
                    op[:],
                    yT_sb[:, P * tb : P * (tb + 1)],
                    wo_sb[:],
                    start=True,
                    stop=True,
                )
                ob = sb_ob.tile([P, CH], bf16, tag="ob", name=f"ob{tb}")
                if last:
                    nc.scalar.copy(ob[:], op[:])
                else:
                    nc.vector.tensor_copy(ob[:], op[:])
                eng = nc.sync if b2 % 2 == 0 else nc.gpsimd
                eng.dma_start(out_d[P * tb : P * (tb + 1), :], ob[:])

            # ---- per-chunk j-loop, STs two steps ahead of yTs (st(c,j) and
            # yT(c,j-2) both unblock on exp(c,j-2), so PE never head-of-line
            # blocks). At each chunk boundary the first two STs of the next
            # chunk are interleaved into the pending drain — their gating
            # events match the drained yTs' exactly, so ACT stays fed
            # through the boundary. tail_dve(c-1) right after the boundary
            # (frees yt banks before yT(c,0)); qkv(c+1)/tail_pe(c-1) later
            # so their PE work sits behind the attention matmuls.
            prev = None  # (chunk, (yt1, yt2)) awaiting tail
            rpair_prev = None
            blocks_left = []
            emit_qkv(0, fast_start=True)
            for c in range(NCH):
                njb = 4 * (c + 1)
                yts = (
                    ps_yt.tile([65, CH], f32, tag="yt", name=f"yt1_{c}"),
                    ps_yt.tile([65, CH], f32, tag="yt", name=f"yt2_{c}"),
                )
                pending = []
                last_c = c == NCH - 1
                for j in range(njb):
                    et, r = emit_st(c, j)
                    pending.append((j, et, r))
                    if j == 1 and prev is not None:
                        rpair_prev = emit_tail_dve(prev[0], prev[1])
                        blocks_left = [0, 1, 2, 3]
                    elif j == 3 and c + 1 < NCH:
                        emit_qkv(c + 1, parts="kq")
                    elif j == 6 and c + 1 < NCH:
                        emit_qkv(c + 1, parts="v")
                    elif j in (5, 7, 9, 11) and prev is not None and blocks_left:
                        emit_tail_block(prev[0], blocks_left.pop(0), rpair_prev)
                        if not blocks_left:
                            prev = None
                    # eager drain at the very end so the final tail starts
                    # as soon as the last exps complete
                    depth = 1 if (last_c and j >= njb - 3) else 2
                    while len(pending) > depth:
                        jj, ee, rr = pending.pop(0)
                        emit_yt(c, jj, ee, rr, yts, jj == 0, jj == njb - 1)
                for jj, ee, rr in pending:
                    emit_yt(c, jj, ee, rr, yts, jj == 0, jj == njb - 1)
                if c == 0 and c + 1 < NCH:
                    emit_qkv(c + 1, parts="v")  # njb=4: no j == 6 slot
                while prev is not None and blocks_left:
                    emit_tail_block(prev[0], blocks_left.pop(0), rpair_prev)
                    if not blocks_left:
                        prev = None
                prev = (c, yts)
            rpair = emit_tail_dve(prev[0], prev[1], last=True)
            for b2 in range(4):
                emit_tail_block(prev[0], b2, rpair, last=True)

    nc.compile()
    return nc


def _host_inputs(x, Wq, Wk, Wv, Wout):
    """Per-core input maps. Core c: batch b=c//4, head-group g=c%4."""
    x = np.asarray(x, dtype=np.float32)
    Wq = np.asarray(Wq, dtype=np.float32)
    Wk = np.asarray(Wk, dtype=np.float32)
    Wv = np.asarray(Wv, dtype=np.float32)
    Wout = np.asarray(Wout, dtype=np.float32)

    # corner mask [128, 2*128]: additive 0/NEG triangular pattern, same for
    # both heads; valid iff col >= row
    col = np.arange(P)[None, :]
    row = np.arange(P)[:, None]
    corner = np.where(col >= row, 0.0, NEG).astype(np.float32)
    mkc = np.tile(corner, (1, 2))
    sel = np.zeros((65, P), dtype=np.float32)
    sel[0, 0:64] = 1.0
    sel[64, 64:P] = 1.0
    idm = np.eye(P, dtype=np.float32)

    def arrange_w(Wc):  # Wc: [128 feat, 512 c] -> lhsT layout [p, (k m)]
        return np.concatenate(
            [np.ascontiguousarray(Wc[:, P * k : P * (k + 1)].T) for k in range(KC)],
            axis=1,
        )

    import ml_dtypes

    bf = ml_dtypes.bfloat16
    in_maps = []
    for core in range(8):
        b, g = core // 4, core % 4
        rows = slice(P * g, P * (g + 1))
        in_maps.append(
            {
                "xT": np.ascontiguousarray(x[b].T).astype(bf),
                "wq": arrange_w(Wq[rows]).astype(bf),
                "wk": arrange_w(Wk[rows]).astype(bf),
                "wv": arrange_w(Wv[rows]).astype(bf),
                "wo": np.ascontiguousarray(Wout[:, rows].T).astype(bf),
                "mkc": mkc,
                "sel": sel,
                "idm": idm,
            }
        )
    return in_maps


def _get_compiled():
    global _COMPILED
    if _COMPILED is None:
        _COMPILED = _build()
    return _COMPILED


def run_on_hw(x, Wq, Wk, Wv, Wout, trace=False):
    """Returns (full_output [B,T,C], exec_time_ns_or_None)."""
    _import_concourse()
    from concourse import bass_utils

    nc = _get_compiled()
    in_maps = _host_inputs(x, Wq, Wk, Wv, Wout)
    res = bass_utils.run_bass_kernel_spmd(
        nc, in_maps, list(range(8)), trace=trace
    )
    global LAST_RESULT
    LAST_RESULT = res
    parts = [res.results[i]["out"].astype(np.float32) for i in range(8)]
    out = np.stack(
        [
            parts[0] + parts[1] + parts[2] + parts[3],
            parts[4] + parts[5] + parts[6] + parts[7],
        ]
    )
    return out, res.exec_time_ns


def kernel(x, Wq, Wk, Wv, Wout):
    out, _ = run_on_hw(x, Wq, Wk, Wv, Wout, trace=False)
    return out


if __name__ == "__main__":
    # smoke test with random data (no reference)
    rng = np.random.default_rng(0)
    x = rng.standard_normal((B, T, C), dtype=np.float32)
    s = 1.0 / np.sqrt(C)
    ws = [rng.standard_normal((C, C), dtype=np.float32) * s for _ in range(4)]
    out = kernel(x, *ws)
    print("out", out.shape, out.dtype, np.abs(out).mean())



# revision 33
# speedup vs baseline: 1.0196x; 1.0196x over previous
"""Causal self-attention Trainium2 Bass kernel (v2).

Problem: B=2, T=4096, C=512, H=8 heads, D=64.
  q = x@Wq.T, k = x@Wk.T, v = x@Wv.T  (per-head split)
  att = softmax(causal(q k^T / sqrt(D)));  y = att @ v;  out = y @ Wout.T

Sharding: 8 cores = 2 batches x 4 head-groups (2 heads each).
Each core computes, for its batch b and heads {2g, 2g+1}:
  - feature-major qT,kT [128, T] bf16 and augmented v blocks via PE matmuls
  - per t1-chunk (512 wide): transposed scores ST[t2_block, t1] = kT^T qT in
    bf16; diagonal blocks stream only the valid causal suffix and get a
    constant 128x128 corner mask added on DVE; exp on ACT (scale=1/8,
    strided suffix AP) into bf16 et tiles; yT_aug[65, t1] accumulation with
    a ones column so row 64 = softmax denominator.
  - tail: denominators -> reciprocal_approx_fast -> broadcast via tiny K=2
    PE matmul, normalize, out = y_norm @ Wout[:, cols]^T -> [T, 512].
  - QKV for chunk c+1 and tail for c-1 are emitted inside chunk c's j-loop
    so the ACT engine (exp, the bottleneck) never drains.
Host sums the 4 partial outputs per batch (row-parallel out projection).
"""

import os
import sys

import numpy as np

B, T, C = 2, 4096, 512
H, D = 8, 64
P = 128          # partitions / t2-block size
CH = 512         # t1 chunk width
NCH = T // CH    # 8 chunks
NTB = T // P     # 32 t-blocks
KC = C // P      # 4 contraction chunks for projections
NEG = -1.0e5     # causal mask additive value (pre-scale)

_COMPILED = None


def _import_concourse():
    try:
        import concourse.bass  # noqa: F401
    except ImportError:
        for p in ("/opt/trn_rl_repo", os.path.expanduser("~/.axon_site/_ro/trn_rl_repo")):
            if os.path.isdir(p) and p not in sys.path:
                sys.path.insert(0, p)
        import concourse.bass  # noqa: F401


def _build():
    """Build + compile the SPMD Bass program (same program on all 8 cores)."""
    _import_concourse()
    import concourse.bass as bass  # noqa: F401
    import concourse.tile as tile
    from concourse import bacc, mybir

    f32 = mybir.dt.float32
    f32r = mybir.dt.float32r
    bf16 = mybir.dt.bfloat16
    EXP = mybir.ActivationFunctionType.Exp

    nc = bacc.Bacc("TRN2", target_bir_lowering=False, debug=False, num_devices=8)

    xT_d = nc.dram_tensor("xT", [C, T], bf16, kind="ExternalInput").ap()
    wq_d = nc.dram_tensor("wq", [P, C], bf16, kind="ExternalInput").ap()
    wk_d = nc.dram_tensor("wk", [P, C], bf16, kind="ExternalInput").ap()
    wv_d = nc.dram_tensor("wv", [P, C], bf16, kind="ExternalInput").ap()
    wo_d = nc.dram_tensor("wo", [P, C], bf16, kind="ExternalInput").ap()
    mkc_d = nc.dram_tensor("mkc", [P, 2 * P], f32, kind="ExternalInput").ap()
    sel_d = nc.dram_tensor("sel", [65, P], f32r, kind="ExternalInput").ap()
    id_d = nc.dram_tensor("idm", [P, P], f32r, kind="ExternalInput").ap()
    out_d = nc.dram_tensor("out", [T, C], bf16, kind="ExternalOutput").ap()

    import contextlib

    with tile.TileContext(nc) as tc, contextlib.ExitStack() as _pctx:
        # ---- persistent SBUF tensors
        persist = _pctx.enter_context(tc.tile_pool(name="persist", bufs=1))

        def ptile(shape, name, dt=f32):
            return persist.tile(shape, dt, name=name, tag=name)

        xT_sb = ptile([P, KC * T], "xT_sb", bf16)      # 4 MB
        wq_sb = ptile([P, C], "wq_sb", bf16)
        wk_sb = ptile([P, C], "wk_sb", bf16)
        wv_sb = ptile([P, C], "wv_sb", bf16)
        wo_sb = ptile([P, C], "wo_sb", bf16)
        mkc_sb = ptile([P, 2 * P], "mkc_sb", f32)
        sel_sb = ptile([65, P], "sel_sb", f32r)
        id_sb = ptile([P, P], "id_sb", f32r)
        qT_sb = ptile([P, T], "qT_sb", bf16)
        kT_sb = ptile([P, T], "kT_sb", bf16)
        va1_sb = ptile([P, NTB * 65], "va1_sb", bf16)
        va2_sb = ptile([P, NTB * 65], "va2_sb", bf16)
        yT_sb = ptile([P, T], "yT_sb", bf16)
        spair_sb = ptile([65, CH], "spair_sb", f32)  # rows 0/64 live, rest 1.0

        # ---- input DMAs ordered by first use: qkv(0) needs wq/wk/wv + xT
        # chunk 0; the first (diagonal) score block needs mkc; id for the v
        # transposes; sel/wo only at the first tail (~30us in).
        def dma_x(c, k):
            eng = nc.sync if (c + k) % 2 == 0 else nc.gpsimd
            eng.dma_start(
                xT_sb[:, T * k + CH * c : T * k + CH * (c + 1)],
                xT_d[P * k : P * (k + 1), CH * c : CH * (c + 1)],
            )

        nc.sync.dma_start(wk_sb[:], wk_d[:])
        nc.gpsimd.dma_start(wq_sb[:], wq_d[:])
        for k in range(KC):
            dma_x(0, k)
        nc.gpsimd.dma_start(wv_sb[:], wv_d[:])
        nc.sync.dma_start(mkc_sb[:], mkc_d[:])
        nc.gpsimd.dma_start(id_sb[:], id_d[:])
        for k in range(KC):
            dma_x(1, k)
        nc.sync.dma_start(sel_sb[:], sel_d[:])
        nc.gpsimd.dma_start(wo_sb[:], wo_d[:])
        for c in range(2, NCH):
            for k in range(KC):
                dma_x(c, k)

        ones_sb = ptile([P, NTB], "ones_sb")  # f32 staging for vaug ones cols
        nc.vector.memset(ones_sb[:], 1.0)
        nc.vector.memset(spair_sb[:], 1.0)
        warm_sb = ptile([P, NTB], "warm_sb")
        nc.scalar.activation(warm_sb[:], ones_sb[:], EXP, scale=0.125)
        ones_src = ones_sb[:].rearrange("p (b s) -> p b s", s=1)
        for va in (va1_sb, va2_sb):
            dst = va[:].rearrange("p (b s) -> p b s", s=65)[:, :, 64:65]
            nc.vector.tensor_copy(dst, ones_src)

        # ---- pools (PSUM: 2*2 + 2*1 + 2*1 = 8 banks)
        with contextlib.ExitStack() as ctx:
            ps_st = ctx.enter_context(tc.tile_pool(name="ps_st", bufs=2, space="PSUM"))
            ps_yt = ctx.enter_context(tc.tile_pool(name="ps_yt", bufs=2, space="PSUM"))
            ps_ms = ctx.enter_context(tc.tile_pool(name="ps_ms", bufs=2, space="PSUM"))
            sb_e = ctx.enter_context(tc.tile_pool(name="sb_e", bufs=8))
            sb_vt = ctx.enter_context(tc.tile_pool(name="sb_vt", bufs=2))
            sb_sm = ctx.enter_context(tc.tile_pool(name="sb_sm", bufs=2))
            sb_ob = ctx.enter_context(tc.tile_pool(name="sb_ob", bufs=6))

            def emit_qkv(c, fast_start=False, parts="kqv"):
                """fast_start (chunk 0 only): compute kT block 0 and qT
                first in narrow pieces so st(0,0) unblocks ~4us earlier
                during the cold-start window. parts selects "kq"/"v" so the
                two halves can be emitted at different points (spreading
                their DVE casts across the chunk)."""
                plan = []
                if "k" in parts:
                    plan += [(wk_sb, "k"), (wq_sb, "q")]
                if "v" in parts:
                    plan += [(wv_sb, "v")]
                if fast_start:
                    plan = [(wk_sb, "k0"), (wq_sb, "q"), (wk_sb, "k1"), (wv_sb, "v")]
                for w_sb, kind in plan:
                    lo, hi = 0, CH
                    if kind == "k0":
                        hi = P
                    elif kind == "k1":
                        lo = P
                    acc = ps_ms.tile([P, CH], f32, tag="ms", name=f"ps_{kind}{c}")
                    for k in range(KC):
                        nc.tensor.matmul(
                            acc[:, lo:hi],
                            w_sb[:, P * k : P * (k + 1)],
                            xT_sb[:, T * k + CH * c + lo : T * k + CH * c + hi],
                            start=(k == 0),
                            stop=(k == KC - 1),
                        )
                    if kind.startswith("k"):
                        nc.vector.tensor_copy(
                            kT_sb[:, CH * c + lo : CH * c + hi], acc[:, lo:hi]
                        )
                    elif kind == "q":
                        nc.vector.tensor_copy(qT_sb[:, CH * c : CH * (c + 1)], acc[:])
                    else:
                        vt = sb_vt.tile([P, CH], f32r, tag="vt", name=f"vt{c}")
                        nc.vector.tensor_copy(vt[:], acc[:])
                        for b2 in range(4):
                            jb = 4 * c + b2
                            tr = ps_ms.tile([P, CH], f32r, tag="ms", name=f"tr{jb}")
                            nc.tensor.transpose(
                                tr[:, 0:P], vt[:, P * b2 : P * (b2 + 1)], id_sb[:]
                            )
                            nc.vector.tensor_copy(
                                va1_sb[:, 65 * jb : 65 * jb + 64], tr[:, 0:64]
                            )
                            nc.vector.tensor_copy(
                                va2_sb[:, 65 * jb : 65 * jb + 64], tr[:, 64:P]
                            )

            def emit_st(c, j):
                """Scores for t2-block j against chunk c's queries.

                Diagonal blocks (j = 4c+r) stream only the valid causal
                suffix [128r:512] and get the constant corner mask added.
                Returns (et tile, r) where r is the suffix offset blocks.
                High priority: these (plus the mask adds and exps) feed ACT,
                the bottleneck engine — the scheduler should always prefer
                them over qkv/tail/yT work when ready.
                """
                with tc.high_priority():
                    return _emit_st(c, j)

            def _emit_st(c, j):
                r = j - 4 * c if j >= 4 * c else 0
                w = CH - P * r  # valid t1 width
                stp = ps_st.tile([P, 2 * CH], f32, tag="st", name=f"st{c}_{j}")
                for h in (0, 1):
                    nc.tensor.matmul(
                        stp[:, CH * h + P * r : CH * (h + 1)],
                        kT_sb[64 * h : 64 * (h + 1), P * j : P * (j + 1)],
                        qT_sb[64 * h : 64 * (h + 1), CH * c + P * r : CH * (c + 1)],
                        start=True,
                        stop=True,
                    )
                if j >= 4 * c:  # diagonal: corner mask on [128, 2, 128]
                    st3 = stp[:].rearrange("p (h w) -> p h w", h=2)
                    nc.vector.tensor_add(
                        st3[:, :, P * r : P * r + P],
                        st3[:, :, P * r : P * r + P],
                        mkc_sb[:].rearrange("p (h w) -> p h w", h=2),
                    )
                et = sb_e.tile([P, 2 * CH], bf16, tag="e", name=f"e{c}_{j}")
                src = stp[:].rearrange("p (h w) -> p h w", h=2)[:, :, P * r :]
                dst = et[:].rearrange("p (h w) -> p h w", h=2)[:, :, P * r :]
                nc.scalar.activation(dst, src, EXP, scale=0.125)
                return et, r

            def emit_yt(c, j, et, r, yts, first, last):
                yt1, yt2 = yts
                for h, yt in ((0, yt1), (1, yt2)):
                    nc.tensor.matmul(
                        yt[:, P * r :],
                        (va1_sb if h == 0 else va2_sb)[:, 65 * j : 65 * j + 65],
                        et[:, CH * h + P * r : CH * (h + 1)],
                        start=first,
                        stop=last,
                    )

            def emit_tail_dve(c, yts, last=False):
                """yt-psum-releasing copies + per-block reciprocals.

                Emitted right after the first ST of the next chunk so the yt
                banks free early (unblocking that chunk's first yT matmul)
                and the reciprocals run on DVE ahead of the qkv casts.
                For the last chunk the copies run on ACT (idle by then).
                """
                yt1, yt2 = yts
                cp = nc.scalar.copy if last else nc.vector.tensor_copy
                cp(spair_sb[0:1, :], yt1[64:65, :])
                cp(spair_sb[64:65, :], yt2[64:65, :])
                cp(yT_sb[0:64, CH * c : CH * (c + 1)], yt1[0:64, :])
                cp(yT_sb[64:P, CH * c : CH * (c + 1)], yt2[0:64, :])
                return sb_sm.tile([65, CH], f32r, tag="rp", name=f"rp{c}")

            def emit_tail_block(c, b2, rpair, last=False):
                """One 128-query block of the tail: reciprocal, denominator
                broadcast, normalize, out projection, drain, DMA. Blocks are
                emitted at spread-out j slots so their DVE work doesn't
                clump and starve the corner-mask adds the exps need. For the
                last chunk the psum drains run on ACT (idle by then)."""
                tb = 4 * c + b2
                sl = slice(P * b2, P * (b2 + 1))
                with nc.allow_low_precision("f32r reciprocal for softmax norm"):
                    nc.vector.reciprocal(rpair[:, sl], spair_sb[:, sl])
                rb = ps_ms.tile([P, P], f32, tag="ms", name=f"rb{tb}")
                nc.tensor.matmul(rb[:], sel_sb[:], rpair[:, sl], start=True, stop=True)
                nc.vector.tensor_mul(
                    yT_sb[:, P * tb : P * (tb + 1)],
                    yT_sb[:, P * tb : P * (tb + 1)],
                    rb[:],
                )
                op = ps_ms.tile([P, CH], f32, tag="ms", name=f"op{tb}")
                nc.tensor.matmul(
                    op[:],
                    yT_sb[:, P * tb : P * (tb + 1)],
                    wo_sb[:],
                    start=True,
                    stop=True,
                )
                ob = sb_ob.tile([P, CH], bf16, tag="ob", name=f"ob{tb}")
                if last:
                    nc.scalar.copy(ob[:], op[:])
                else:
                    nc.vector.tensor_copy(ob[:], op[:])
                eng = nc.sync if b2 % 2 == 0 else nc.gpsimd
                eng.dma_start(out_d[P * tb : P * (tb + 1), :], ob[:])

            # ---- per-chunk j-loop, STs two steps ahead of yTs (st(c,j) and
            # yT(c,j-2) both unblock on exp(c,j-2), so PE never head-of-line
            # blocks). At each chunk boundary the first two STs of the next
            # chunk are interleaved into the pending drain — their gating
            # events match the drained yTs' exactly, so ACT stays fed
            # through the boundary. tail_dve(c-1) right after the boundary
            # (frees yt banks before yT(c,0)); qkv(c+1)/tail_pe(c-1) later
            # so their PE work sits behind the attention matmuls.
            prev = None  # (chunk, (yt1, yt2)) awaiting tail
            rpair_prev = None
            blocks_left = []
            emit_qkv(0, fast_start=True)
            for c in range(NCH):
                njb = 4 * (c + 1)
                yts = (
                    ps_yt.tile([65, CH], f32, tag="yt", name=f"yt1_{c}"),
                    ps_yt.tile([65, CH], f32, tag="yt", name=f"yt2_{c}"),
                )
                pending = []
                last_c = c == NCH - 1
                for j in range(njb):
                    et, r = emit_st(c, j)
                    pending.append((j, et, r))
                    if c == 0:
                        # chunk 0 is short and input-DMA-gated: pull the
                        # next chunk's projections as early as possible
                        if j == 1:
                            emit_qkv(1, parts="kq")
                        elif j == 3:
                            emit_qkv(1, parts="v")
                    elif j == 1:
                        rpair_prev = emit_tail_dve(prev[0], prev[1])
                        blocks_left = [0, 1, 2, 3]
                    elif j == 3 and c + 1 < NCH:
                        emit_qkv(c + 1, parts="kq")
                    elif j == 6 and c + 1 < NCH:
                        emit_qkv(c + 1, parts="v")
                    elif j == 5 and prev is not None:
                        for b2 in blocks_left:
                            emit_tail_block(prev[0], b2, rpair_prev)
                        blocks_left = []
                        prev = None
                    # eager drain at the very end so the final tail starts
                    # as soon as the last exps complete
                    depth = 1 if (last_c and j >= njb - 3) else 2
                    while len(pending) > depth:
                        jj, ee, rr = pending.pop(0)
                        emit_yt(c, jj, ee, rr, yts, jj == 0, jj == njb - 1)
                for jj, ee, rr in pending:
                    emit_yt(c, jj, ee, rr, yts, jj == 0, jj == njb - 1)
                prev = (c, yts)
            rpair = emit_tail_dve(prev[0], prev[1], last=True)
            for b2 in range(4):
                emit_tail_block(prev[0], b2, rpair, last=True)

    nc.compile()
    return nc


def _host_inputs(x, Wq, Wk, Wv, Wout):
    """Per-core input maps. Core c: batch b=c//4, head-group g=c%4."""
    x = np.asarray(x, dtype=np.float32)
    Wq = np.asarray(Wq, dtype=np.float32)
    Wk = np.asarray(Wk, dtype=np.float32)
    Wv = np.asarray(Wv, dtype=np.float32)
    Wout = np.asarray(Wout, dtype=np.float32)

    # corner mask [128, 2*128]: additive 0/NEG triangular pattern, same for
    # both heads; valid iff col >= row
    col = np.arange(P)[None, :]
    row = np.arange(P)[:, None]
    corner = np.where(col >= row, 0.0, NEG).astype(np.float32)
    mkc = np.tile(corner, (1, 2))
    sel = np.zeros((65, P), dtype=np.float32)
    sel[0, 0:64] = 1.0
    sel[64, 64:P] = 1.0
    idm = np.eye(P, dtype=np.float32)

    def arrange_w(Wc):  # Wc: [128 feat, 512 c] -> lhsT layout [p, (k m)]
        return np.concatenate(
            [np.ascontiguousarray(Wc[:, P * k : P * (k + 1)].T) for k in range(KC)],
            axis=1,
        )

    import ml_dtypes

    bf = ml_dtypes.bfloat16
    in_maps = []
    for core in range(8):
        b, g = core // 4, core % 4
        rows = slice(P * g, P * (g + 1))
        in_maps.append(
            {
                "xT": np.ascontiguousarray(x[b].T).astype(bf),
                "wq": arrange_w(Wq[rows]).astype(bf),
                "wk": arrange_w(Wk[rows]).astype(bf),
                "wv": arrange_w(Wv[rows]).astype(bf),
                "wo": np.ascontiguousarray(Wout[:, rows].T).astype(bf),
                "mkc": mkc,
                "sel": sel,
                "idm": idm,
            }
        )
    return in_maps


def _get_compiled():
    global _COMPILED
    if _COMPILED is None:
        _COMPILED = _build()
    return _COMPILED


def run_on_hw(x, Wq, Wk, Wv, Wout, trace=False):
    """Returns (full_output [B,T,C], exec_time_ns_or_None)."""
    _import_concourse()
    from concourse import bass_utils

    nc = _get_compiled()
    in_maps = _host_inputs(x, Wq, Wk, Wv, Wout)
    res = bass_utils.run_bass_kernel_spmd(
        nc, in_maps, list(range(8)), trace=trace
    )
    global LAST_RESULT
    LAST_RESULT = res
    parts = [res.results[i]["out"].astype(np.float32) for i in range(8)]
    out = np.stack(
        [
            parts[0] + parts[1] + parts[2] + parts[3],
            parts[4] + parts[5] + parts[6] + parts[7],
        ]
    )
    return out, res.exec_time_ns


def kernel(x, Wq, Wk, Wv, Wout):
    out, _ = run_on_hw(x, Wq, Wk, Wv, Wout, trace=False)
    return out


if __name__ == "__main__":
    # smoke test with random data (no reference)
    rng = np.random.default_rng(0)
    x = rng.standard_normal((B, T, C), dtype=np.float32)
    s = 1.0 / np.sqrt(C)
    ws = [rng.standard_normal((C, C), dtype=np.float32) * s for _ in range(4)]
    out = kernel(x, *ws)
    print("out", out.shape, out.dtype, np.abs(out).mean())

